# revision 75
# baseline (speedup 1.0000x reference)
"""Matryoshka soft-top-k gating kernel for Trainium2 (Bass/Tile).

Computes, for each matryoshka scale k in (128, 64, 32):
    scores  = emb @ w  (+ b, which cancels in scores - threshold)
    scores  = where(mask, scores, -BIG)
    thr_k   = k-th largest score per row
    diff    = min(scores - thr_k, CLAMP)       (lower clip can't bind;
              masked -BIG diffs saturate the sigmoid to exactly 0, so the
              post-sigmoid mask multiply is folded away)
    gate    = sigmoid(diff * temperature)
    out_k   = emb * gate[..., None]

Sharding: data-parallel over the batch axis across 8 NeuronCores
(64 rows per core); w/temperature replicated, mask sharded with batch.

HBM traffic is the roofline, so embeddings are uploaded as fp16
pre-scaled by 2^10 (the scale dodges the fp16 subnormal cliff; scores
scale out in the sigmoid's temperature and the outputs are unscaled on
the host) and outputs are stored as bf16 (combined rel err ~1.4e-2 vs
the 2e-2 tolerance).  Both live in device layouts whose innermost
contiguous runs are >=2KB: <512B DMA descriptors cost 2x bandwidth.
The host pre-transposes embeddings into token-major [128, NT*D] and
reassembles/unscales the bf16 outputs.

Per-core pipeline: scores are computed on PE (transpose each 128-token
tile against an fp16 identity, batch-copy the transposed chunk to SBUF
split across ACT and DVE, then matvec against w) so DVE stays free for
the serial threshold chains.  A max8+match_replace chain round costs
free-size cycles regardless of partition count, so two narrow 8-round
chains (rows 0-15, 16-31) run inside the load window and feed the first
four stores (k=32/64), while a single full-rows 16-round chain supplies
all remaining thresholds.  Gating is spread over DVE (merged
tensor_tensor groups against interleaved gate-column broadcasts), Pool
and ACT (per-tile activation with an f32 scale column), and the twelve
(k, 16-row) stores are emitted in production order so the store stream
follows the input load with the DMA engines near-continuously busy.
"""

import numpy as np

import concourse.bacc as bacc
import concourse.bass as bass
import concourse.mybir as mybir
import concourse.tile as tile
from concourse.bass_utils import run_bass_kernel_spmd

N_CORES = 8
B, T, D = 512, 256, 128
R = B // N_CORES          # rows (documents) per core
KS = (128, 64, 32)
CLAMP = 50.0
BIG = 3.4e38              # stands in for -inf in masked_fill
REPL = -3.0e38            # match_replace sentinel (> -BIG)
NT = R * T // 128         # 128-token tiles per core (128)
CH = 16                   # tiles per DMA chunk
NCH = NT // CH            # 8 chunks
UNIT_ROWS = 16            # rows per store unit
UNIT_TILES = UNIT_ROWS * 2          # 32 tiles per store unit
N_UNITS = R // UNIT_ROWS            # 4

# ---- variant switch -------------------------------------------------------
# "f32": embeddings uploaded f32 (safe, ~4e-3 rel err)
# "f16": embeddings uploaded fp16 pre-scaled by 2^10 (~1.4e-2 rel err)
VARIANT = "f16"

f32 = mybir.dt.float32
f16 = mybir.dt.float16
bf16 = mybir.dt.bfloat16
u8 = mybir.dt.uint8
Alu = mybir.AluOpType
Act = mybir.ActivationFunctionType

if VARIANT == "f32":
    EMB_DT = f32
    EMB_NP = np.float32
    EMB_SCALE = 1.0
else:
    EMB_DT = f16
    EMB_NP = np.float16
    EMB_SCALE = 1024.0

# chunk scores: n on DVE, rest on Pool
SCORE_DVE = 5

# Chains: a round costs free-size cycles only (independent of partition
# count), so two cheap 8-round chains on rows 0-15 / 16-31 run inside the
# otherwise-idle load window and feed the first four stores, while the
# single full-rows 16-round chain (starting the moment the last chunk is
# scored) supplies everything else.
SCOPES = {
    "A": (0, 16, 8),
    "B": (16, 32, 8),
    "F": (0, 64, 16),
}
# store plan: ordered (k, unit, scope) — scope supplies the threshold.
STORE_PLAN = [
    (32, 0, "A"), (32, 1, "B"),
    (64, 0, "A"), (64, 1, "B"),
    (32, 2, "F"), (32, 3, "F"),
    (64, 2, "F"), (64, 3, "F"),
    (128, 0, "F"), (128, 1, "F"), (128, 2, "F"), (128, 3, "F"),
]  # slots run ~2.9us apart once the input load drains
# engine pattern per store unit: 4 groups of 8 tiles -> D(VE tt8/tt16),
# P(ool tt8), A(CT per-tile).  Adjacent D groups merge into one wide op.
# A-side rides Pool (idle early), mid units Pool+ACT during the F-chain,
# k=128 units ride DVE (free after the chain).
UNIT_ENGINES = {
    (32, 0): "PAPA", (32, 1): "APAP",
    (64, 0): "PPAA", (64, 1): "PAPA",
    (32, 2): "PAPA", (32, 3): "APAP",
    (64, 2): "PAPA", (64, 3): "APPA",
    (128, 0): "DDDD", (128, 1): "DDDD", (128, 2): "DDPA", (128, 3): "DPAA",
}


def build_bass():
    nc = bacc.Bacc("TRN2", target_bir_lowering=False, debug=False)

    emb = nc.dram_tensor("emb_tm", [128, NT * D], EMB_DT, kind="ExternalInput")
    w = nc.dram_tensor("w", [D], f32, kind="ExternalInput")
    temp = nc.dram_tensor("temperature", [1], f32, kind="ExternalInput")
    mask = nc.dram_tensor("mask", [R, T], u8, kind="ExternalInput")
    out = nc.dram_tensor("out", [len(KS), 128, NT * D], bf16,
                         kind="ExternalOutput")
    ident_d = nc.inline_tensor(np.eye(128, dtype=np.float32), name="ident128")

    with tile.TileContext(nc) as tc:
        with (
            tc.tile_pool(name="singles", bufs=1) as singles,
            tc.tile_pool(name="out0", bufs=8) as opool,
            tc.tile_pool(name="ebt", bufs=3) as ebtpool,
            tc.tile_pool(name="psum", bufs=2, space="PSUM") as psum,
        ):
            st = _State(nc, singles, psum, opool, ebtpool, out)

            # ---- input loads first so DMA ramps immediately ----
            embbuf = singles.tile([128, NT * D], EMB_DT)
            st.embbuf = embbuf

            def load_chunk(ci):
                nc.sync.dma_start(
                    out=embbuf[:, ci * CH * D:(ci + 1) * CH * D],
                    in_=emb.ap()[:, ci * CH * D:(ci + 1) * CH * D],
                )

            ident = singles.tile([128, 128], f32)
            nc.sync.dma_start(out=ident, in_=ident_d.ap())
            st.ident = ident
            load_chunk(0)
            w_stage = singles.tile([128, D], f32)
            nc.sync.dma_start(out=w_stage, in_=_bcast(w.ap(), 128))
            load_chunk(1)
            w_col_stage = singles.tile([128, 1], f32)
            nc.sync.dma_start(
                out=w_col_stage,
                in_=bass.AP(tensor=w.ap().tensor, offset=0,
                            ap=[[1, 128], [0, 1]]),
            )
            load_chunk(2)
            for s, (lo, hi, _) in SCOPES.items():
                rg = hi - lo
                mus = singles.tile([rg, T], u8, tag=f"mu{s}", name=f"mu{s}")
                nc.sync.dma_start(out=mus, in_=mask.ap()[lo:hi, :])
                st.mask_u[s] = mus
            for s, (lo, hi, _) in SCOPES.items():
                rg = hi - lo
                tcs = singles.tile([rg, 1], f32, tag=f"tc{s}", name=f"tc{s}")
                nc.sync.dma_start(out=tcs, in_=_bcast(temp.ap(), rg))
                st.temp_col[s] = tcs
            load_chunk(3)
            for ci in range(4, NCH):
                load_chunk(ci)

            # ---- SBUF constants ----
            w_col = singles.tile([128, 1], EMB_DT)
            nc.vector.tensor_copy(w_col, w_col_stage)
            st.w_col = w_col
            w_rep = singles.tile([128, D], EMB_DT)
            nc.vector.tensor_copy(w_rep, w_stage)
            st.w_rep = w_rep
            if EMB_DT is not f32:
                ident16 = singles.tile([128, 128], EMB_DT)
                nc.vector.tensor_copy(ident16, ident)
                st.ident16 = ident16
            else:
                st.ident16 = ident
            if EMB_SCALE != 1.0:
                for s in SCOPES:
                    nc.gpsimd.tensor_scalar_mul(
                        st.temp_col[s], st.temp_col[s], 1.0 / EMB_SCALE)
            sig_warm = singles.tile([1, 1], f32)
            nc.scalar.activation(sig_warm, st.temp_col["A"][0:1, :],
                                 Act.Sigmoid, bias=0.0, scale=1.0)

            # PE p-state warmup: keep PE busy before the first chunk lands
            # so transposes run at full clock (p-state HIGH needs ~3us).
            pwarm = st.psum.tile([128, 128], f32, tag="pst", name="pwarm",
                                 bufs=2)
            for _ in range(10):
                nc.tensor.transpose(pwarm, ident, ident)

            st.scores_tm = singles.tile([128, NT], f32)
            st.trash_d = singles.tile([128, 1], EMB_DT)
            st.trash_p = singles.tile([128, 1], EMB_DT)

            _emit_pipeline(st)

    nc.compile()
    return nc


def _bcast(ap, n):
    """Replicate a DRAM vector across n partitions (0-step leading dim)."""
    return bass.AP(tensor=ap.tensor, offset=ap.offset, ap=[[0, n]] + list(ap.ap))


class _State:
    def __init__(self, nc, singles, psum, opool, ebtpool, out):
        self.nc = nc
        self.singles = singles
        self.psum = psum
        self.opool = opool
        self.ebtpool = ebtpool
        self.out = out
        self.embbuf = None
        self.ident = None
        self.ident16 = None
        self.w_rep = None
        self.w_col = None
        self.mask_u = {}
        self.temp_col = {}
        self.mask_f = {}
        self.fillt = {}
        self.scores_tm = None
        self.trash_d = None
        self.trash_p = None
        self.scores_e = None
        self.scores_o = None
        self.srm = {}
        self.work = {}
        self.rounds_done = {}
        self.last_mx = {}
        self.gcols = {}
        self.ebts = {}
        self.ochs = {}

    def mark(self, label):
        PHASES.append((label, self.nc.next_id()))

    def tile_ap(self, t):
        return self.embbuf[:, t * D:(t + 1) * D]

    def och_ap(self, och, t_local, n):
        return och[:, t_local * D:(t_local + n) * D]


NPE = 16  # tiles per chunk scored on PE


def _score_transpose(st, ci):
    """PE-transpose 6 of the chunk's tiles to PSUM, start the ebT copy,
    and score the remaining 2 tiles with Pool fused multiply+reduce."""
    st.mark(f'scoreT_c{ci}')
    nc = st.nc
    ptile = st.psum.tile([128, NPE * 128], EMB_DT, tag="ptile", name=f"pt{ci}",
                         bufs=2)
    for j in range(NPE):
        t = ci * CH + j
        nc.tensor.transpose(ptile[:, j * 128:(j + 1) * 128], st.tile_ap(t),
                            st.ident16)
    ebT = st.ebtpool.tile([128, NPE * 128], EMB_DT, tag="ebT")
    half = NPE * 64
    nc.scalar.copy(ebT[:, 0:half], ptile[:, 0:half])
    nc.vector.tensor_copy(ebT[:, half:], ptile[:, half:])
    st.ebts[ci] = ebT


def _score_matvec(st, ci):
    """Matvec a transposed chunk against w_col; copy the score columns out."""
    st.mark(f'scoreM_c{ci}')
    nc = st.nc
    ebT = st.ebts[ci]
    psc = st.psum.tile([128, NPE], f32, tag="psc", name=f"psc{ci}", bufs=2)
    for j in range(NPE):
        nc.tensor.matmul(psc[:, j:j + 1], ebT[:, j * 128:(j + 1) * 128],
                         st.w_col)
    nc.vector.tensor_copy(st.scores_tm[:, ci * CH:ci * CH + NPE], psc)


def _score_chunk(st, ci):
    """Software-pipelined: transposes of chunk ci, matvecs of chunk ci-1 —
    one chunk of lag absorbs the PE->ACT->PE copy latency."""
    _score_transpose(st, ci)
    if ci >= 1:
        _score_matvec(st, ci - 1)
    if ci == NCH - 1:
        _score_matvec(st, ci)


def _deinterleave(st, row_lo, row_hi):
    st.mark(f'deint_{row_lo}_{row_hi}')
    nc = st.nc
    if st.scores_e is None:
        st.scores_e = st.singles.tile([128, R], f32)
        st.scores_o = st.singles.tile([128, R], f32)
    n = row_hi - row_lo
    src = st.scores_tm
    se = bass.AP(tensor=src.tensor, offset=src.offset + 2 * row_lo,
                 ap=[list(src.ap[0]), [2, n]])
    so = bass.AP(tensor=src.tensor, offset=src.offset + 2 * row_lo + 1,
                 ap=[list(src.ap[0]), [2, n]])
    nc.vector.tensor_copy(st.scores_e[:, row_lo:row_hi], se)
    nc.vector.tensor_copy(st.scores_o[:, row_lo:row_hi], so)


def _prefix(st, scope):
    """Masked row-major scores for a scope: transpose + mask fill.
    The scope's mask/fill tiles convert just-in-time on Pool."""
    st.mark(f'prefix_{scope}')
    nc = st.nc
    lo, hi, _ = SCOPES[scope]
    rg = hi - lo
    mfs = st.singles.tile([rg, T], f32, tag=f"mf{scope}", name=f"mf{scope}")
    nc.gpsimd.tensor_copy(mfs, st.mask_u[scope])
    ft = st.singles.tile([rg, T], f32, tag=f"ft{scope}", name=f"ft{scope}")
    nc.gpsimd.tensor_scalar(
        out=ft, in0=mfs, scalar1=1.0, scalar2=BIG,
        op0=Alu.subtract, op1=Alu.mult,
    )
    st.mask_f[scope] = mfs
    st.fillt[scope] = ft
    srm = st.singles.tile([rg, T], f32, tag=f"srm{scope}", name=f"srm{scope}")
    pse = st.psum.tile([rg, 128], f32, tag="psc", name=f"pse{scope}", bufs=2)
    nc.tensor.transpose(pse, st.scores_e[:, lo:hi], st.ident)
    nc.vector.tensor_copy(srm[:, 0:128], pse)
    pso = st.psum.tile([rg, 128], f32, tag="psc", name=f"pso{scope}", bufs=2)
    nc.tensor.transpose(pso, st.scores_o[:, lo:hi], st.ident)
    nc.vector.tensor_copy(srm[:, 128:256], pso)
    msk = st.singles.tile([rg, T], f32, tag=f"smsk{scope}",
                          name=f"smsk{scope}")
    nc.vector.scalar_tensor_tensor(
        out=msk, in0=srm, scalar=1.0, in1=st.mask_f[scope],
        op0=Alu.mult, op1=Alu.mult,
    )
    nc.vector.tensor_add(msk, msk, st.fillt[scope])
    st.srm[scope] = msk
    st.work[scope] = st.singles.tile([rg, T], f32, tag=f"work{scope}",
                                     name=f"work{scope}")
    st.rounds_done[scope] = 0


def _rounds(st, scope, upto):
    st.mark(f'rounds_{scope}_{upto}')
    nc = st.nc
    lo, hi, n_rounds = SCOPES[scope]
    rg = hi - lo
    while st.rounds_done[scope] < upto:
        r = st.rounds_done[scope]
        mx = st.singles.tile([rg, 8], f32, tag=f"mx{scope}_{r}",
                             name=f"mx{scope}_{r}")
        src = st.srm[scope] if r == 0 else st.work[scope]
        nc.vector.max(out=mx, in_=src)
        if r < n_rounds - 1:
            nc.vector.match_replace(
                out=st.work[scope], in_to_replace=mx, in_values=src,
                imm_value=REPL,
            )
        st.last_mx[scope] = mx
        st.rounds_done[scope] += 1
    return st.last_mx[scope][:, 7:8]


def _make_gcols(st, scope, k, thr):
    """diff -> sigmoid -> transpose into interleaved gate cols [128, 2*rg]."""
    st.mark(f'gcols_{scope}_{k}')
    nc = st.nc
    lo, hi, _ = SCOPES[scope]
    rg = hi - lo
    dif = st.singles.tile([rg, T], f32, tag=f"dif{scope}_{k}",
                          name=f"dif{scope}_{k}")
    nc.gpsimd.tensor_scalar(
        out=dif, in0=st.srm[scope], scalar1=thr, scalar2=CLAMP * EMB_SCALE,
        op0=Alu.subtract, op1=Alu.min,
    )
    nc.scalar.activation(dif, dif, Act.Sigmoid, bias=0.0,
                         scale=st.temp_col[scope])
    g = st.singles.tile([128, 2 * rg], f32, tag=f"g{scope}_{k}",
                        name=f"g{scope}_{k}")
    pme = st.psum.tile([128, rg], f32, tag="pst", name=f"pme{scope}{k}",
                       bufs=2)
    nc.tensor.transpose(pme, dif[:, 0:128], st.ident[:rg, :rg])
    ge = bass.AP(tensor=g.tensor, offset=g.offset, ap=[list(g.ap[0]), [2, rg]])
    nc.vector.tensor_copy(ge, pme)
    pmo = st.psum.tile([128, rg], f32, tag="pst", name=f"pmo{scope}{k}",
                       bufs=2)
    nc.tensor.transpose(pmo, dif[:, 128:256], st.ident[:rg, :rg])
    go = bass.AP(tensor=g.tensor, offset=g.offset + 1,
                 ap=[list(g.ap[0]), [2, rg]])
    nc.vector.tensor_copy(go, pmo)
    st.gcols[(scope, k)] = g


def _gate_unit(st, k, unit, scope, split_store=False):
    st.mark(f'gate_{k}_{unit}')
    nc = st.nc
    lo, hi, _ = SCOPES[scope]
    g = st.gcols[(scope, k)]
    och = st.opool.tile([128, UNIT_TILES * D], bf16, tag="och")
    st.ochs[(k, unit)] = och
    t0 = unit * UNIT_TILES
    pat = UNIT_ENGINES[(k, unit)]
    k_i = KS.index(k)
    gi = 0
    while gi < 4:
        eng = pat[gi]
        ngr = 1
        if eng == "D":
            while gi + ngr < 4 and pat[gi + ngr] == eng:
                ngr += 1
        ts = t0 + gi * 8
        gofs = ts - 2 * lo
        if eng == "A":
            for j in range(8):
                t = ts + j
                col = g[:, gofs + j:gofs + j + 1]
                nc.scalar.activation(
                    st.och_ap(och, t - t0, 1), st.tile_ap(t), Act.Copy,
                    bias=0.0, scale=col,
                )
        else:
            n = ngr * 8
            gb = bass.AP(tensor=g.tensor, offset=g.offset + gofs,
                         ap=[list(g.ap[0]), [1, n], [0, D]])
            e = nc.vector if eng == "D" else nc.gpsimd
            e.tensor_tensor(
                out=st.och_ap(och, ts - t0, n),
                in0=st.embbuf[:, ts * D:(ts + n) * D],
                in1=gb, op=Alu.mult,
            )
        if split_store:
            # ship each gated 8-tile group immediately (2KB descriptors,
            # still full DMA bandwidth; HWDGE has slack)
            n = ngr * 8
            nc.sync.dma_start(
                out=st.out.ap()[k_i, :, ts * D:(ts + n) * D],
                in_=st.och_ap(och, ts - t0, n),
            )
        gi += ngr


def _store_unit(st, k, unit):
    st.mark(f'store_{k}_{unit}')
    nc = st.nc
    k_i = KS.index(k)
    nc.sync.dma_start(
        out=st.out.ap()[k_i, :,
                        unit * UNIT_TILES * D:(unit + 1) * UNIT_TILES * D],
        in_=st.ochs[(k, unit)],
    )


def _emit_pipeline(st):
    _score_chunk(st, 0)
    _score_chunk(st, 1)
    _score_chunk(st, 2)
    # scope A (rows 0-15 = chunks 0-1)
    _deinterleave(st, 0, 16)
    _prefix(st, "A")
    thrA32 = _rounds(st, "A", 4)
    _make_gcols(st, "A", 32, thrA32)
    _score_chunk(st, 3)
    _gate_unit(st, 32, 0, "A", split_store=True)
    _score_chunk(st, 4)
    # scope B (rows 16-31 = chunks 2-3)
    _deinterleave(st, 16, 32)
    _prefix(st, "B")
    thrB32 = _rounds(st, "B", 4)
    _make_gcols(st, "B", 32, thrB32)
    _score_chunk(st, 5)
    _gate_unit(st, 32, 1, "B", split_store=True)
    _score_chunk(st, 6)
    thrA64 = _rounds(st, "A", 8)
    _make_gcols(st, "A", 64, thrA64)
    _gate_unit(st, 64, 0, "A", split_store=True)
    _score_chunk(st, 7)
    thrB64 = _rounds(st, "B", 8)
    _make_gcols(st, "B", 64, thrB64)
    _gate_unit(st, 64, 1, "B", split_store=True)
    # full chain over all 64 rows supplies every remaining threshold
    _deinterleave(st, 32, 64)
    _prefix(st, "F")
    thrF32 = _rounds(st, "F", 4)
    _make_gcols(st, "F", 32, thrF32)
    _gate_unit(st, 32, 2, "F")
    _store_unit(st, 32, 2)
    _gate_unit(st, 32, 3, "F")
    _store_unit(st, 32, 3)
    thrF64 = _rounds(st, "F", 8)
    _make_gcols(st, "F", 64, thrF64)
    _gate_unit(st, 64, 2, "F")
    _store_unit(st, 64, 2)
    _gate_unit(st, 64, 3, "F")
    _store_unit(st, 64, 3)
    thrF128 = _rounds(st, "F", 16)
    _make_gcols(st, "F", 128, thrF128)
    for u in range(4):
        _gate_unit(st, 128, u, "F", split_store=True)


PHASES = []


_NC = None


def _get_nc():
    global _NC
    if _NC is None:
        _NC = build_bass()
    return _NC


def make_in_maps(embeddings, w, temperature, mask):
    """Shard + device-layout the full inputs for the 8 cores."""
    emb = np.asarray(embeddings, dtype=np.float32)
    w = np.ascontiguousarray(np.asarray(w, dtype=np.float32))
    temp = np.ascontiguousarray(np.asarray(temperature, dtype=np.float32))
    mask_u8 = np.asarray(mask).astype(np.uint8)
    in_maps = []
    for c in range(N_CORES):
        sl = slice(c * R, (c + 1) * R)
        esh = emb[sl].reshape(NT, 128, D).transpose(1, 0, 2).reshape(128, NT * D)
        if EMB_SCALE != 1.0:
            esh = esh * EMB_SCALE
        in_maps.append({
            "emb_tm": np.ascontiguousarray(esh.astype(EMB_NP)),
            "w": w,
            "temperature": temp,
            "mask": np.ascontiguousarray(mask_u8[sl]),
        })
    return in_maps


def postprocess(results):
    """Device bf16 [3, 128, NT*D] outputs -> full [3, B, T, D] f32."""
    outs = []
    for r in results:
        o = np.asarray(r["out"]).astype(np.float32)
        if EMB_SCALE != 1.0:
            o *= 1.0 / EMB_SCALE
        o = o.reshape(len(KS), 128, NT, D).transpose(0, 2, 1, 3)
        outs.append(o.reshape(len(KS), R, T, D))
    return np.concatenate(outs, axis=1)


def kernel(embeddings, w, b, temperature, mask):
    nc = _get_nc()
    in_maps = make_in_maps(embeddings, w, temperature, mask)
    res = run_bass_kernel_spmd(nc, in_maps, core_ids=list(range(N_CORES)))
    return postprocess(res.results)
